# revision 1
# baseline (speedup 1.0000x reference)
"""Trainium2 Bass kernel for a 2-layer GAT + global-mean-pool + FC model.

Strategy (8 NeuronCores, SPMD):
  - Nodes are partitioned across cores at graph boundaries (32 graphs/core),
    padded to NLOC rows per core; "padded row id" space is the concatenation
    of all cores' padded segments (PROWS rows total).
  - GAT layer aggregation is linear in the source features, so layer 1
    aggregates the *74-dim inputs* (not the 1024-dim hidden vectors):
      out1[dst,h] = (sum_e alpha_eh * x[src_e]) @ W1_h
    which cuts edge-gather traffic ~14x.
  - Each core builds a per-node feature table (xext: [x | 1 | al_src]),
    AllGathers it, then processes the edges whose dst it owns:
    per 128-dst-node block, per 128-edge chunk:
      * DMA-gather the source rows,
      * build the one-hot edge->dst matrix M^T on the vector engine
        (is_equal against an iota row),
      * attention logits: al_src from the gathered row + al_dst via a tiny
        PE matmul (M @ al_dst_block); exp(leakyrelu(z)) = max(e^z, e^.2z),
      * scale gathered rows by exp-weights, matmul-accumulate into PSUM;
        a constant 1.0 column yields the softmax denominator for free,
      * normalize by the denominator at block end.
  - Layer 2 runs the same machinery over a [h2 | 1 | al_src2] table
    (h2 = relu(out1) @ W2 computed locally, AllGathered).
  - Pooling is a one-hot (node->graph) matmul accumulated over blocks;
    the ones column yields node counts. Final FC on-chip, [32,12] per core.

All per-core variation travels through input tensors (SPMD: one program).
"""

import math
import os
import sys

sys.path.insert(0, "/opt/trn_rl_repo")

import numpy as np
import ml_dtypes

import concourse.bass as bass
import concourse.bacc as bacc
import concourse.mybir as mybir
import concourse.tile as tile
from concourse.bass_utils import run_bass_kernel_spmd

BF16 = mybir.dt.bfloat16
F32 = mybir.dt.float32
I16 = mybir.dt.int16

NEG_SLOPE = 0.2

# ---------------------------------------------------------------------------
# Model dims (problem constants)
N_NODES = 50000
N_EDGES = 200000
NODE_DIM = 74
HIDDEN = 256
HEADS = 4
OUT_DIM = 12
N_GRAPHS = 256
N_CORES = 8

# xext row: [x(74) | 1.0 | al_s(HEADS) | pad] in bf16, padded to XW cols
XW = 128
COL_ONE = NODE_DIM          # 74
COL_ALS = NODE_DIM + 1      # 75
AGG_W = NODE_DIM + 1        # 75: matmul rhs slice [x | 1]

# h2ext row: [h2(256) | 1.0 | al_s2 | pad] in bf16, padded to HW2 cols
HW2 = 384
H_COL_ONE = HIDDEN          # 256
H_COL_ALS = HIDDEN + 1      # 257
H_AGG_W = HIDDEN + 2        # 258: rhs slice [h2 | 1 | al_s2] (last col unused out)

LO_LIMIT = 28672  # int16 gather index limit (values near 32767 fault the ucode)
MAX_GATHER_CHUNKS = 8  # cap descriptors per dma_gather call (1024 rows)


class Layout:
    """Static (core-uniform) layout computed on the host from the edge data."""

    def __init__(self, n_nodes, n_graphs, n_cores, edges_src, edges_dst, batch,
                 l1_group_blocks=16, l2_group_blocks=8):
        self.n_cores = n_cores
        g_per_core = n_graphs // n_cores
        assert g_per_core * n_cores == n_graphs
        gb = np.searchsorted(batch, np.arange(n_graphs + 1))
        self.core_start = gb[np.arange(n_cores) * g_per_core]
        self.core_end = gb[(np.arange(n_cores) + 1) * g_per_core]
        n_local = self.core_end - self.core_start
        self.NLOC = int(math.ceil(n_local.max() / 128) * 128)
        self.NB = self.NLOC // 128
        self.PROWS = self.NLOC * n_cores
        assert self.PROWS <= 2 * LO_LIMIT, "lo/hi gather windows must cover all rows"
        self.HI_BASE = self.PROWS - LO_LIMIT if self.PROWS > LO_LIMIT else 0
        self.g_per_core = g_per_core

        # node -> (core, padded row)
        core_of = np.searchsorted(self.core_end, np.arange(n_nodes), side="right")
        prow = self.NLOC * core_of + (np.arange(n_nodes) - self.core_start[core_of])
        self.prow = prow

        dst_core = core_of[edges_dst]
        dstloc = edges_dst - self.core_start[dst_core]
        blk = dstloc // 128
        # per (core, block): lo/hi edge lists
        src_p = prow[edges_src]
        is_lo = src_p < LO_LIMIT

        self.edges = []  # per core: dict(block -> (lo_idx_array, hi_idx_array)) of edge ids
        nlo = np.zeros((n_cores, self.NB), dtype=np.int64)
        nhi = np.zeros((n_cores, self.NB), dtype=np.int64)
        for c in range(n_cores):
            sel = np.nonzero(dst_core == c)[0]
            per_block = {}
            bsel = blk[sel]
            for b in range(self.NB):
                e_b = sel[bsel == b]
                lo_e = e_b[is_lo[e_b]]
                hi_e = e_b[~is_lo[e_b]]
                per_block[b] = (lo_e, hi_e)
                nlo[c, b] = len(lo_e)
                nhi[c, b] = len(hi_e)
            self.edges.append(per_block)

        self.Klo = np.maximum(np.ceil(nlo.max(axis=0) / 128), 0).astype(int)
        self.Khi = np.maximum(np.ceil(nhi.max(axis=0) / 128), 0).astype(int)

        # groups: list of (block_ids, chunks) where chunks is an ordered list of
        # (block, kind) per 128-edge chunk; order = all lo chunks (by block),
        # then all hi chunks (by block). Each group does <=2 dma_gather calls.
        def make_groups(gsz):
            groups = []
            for s in range(0, self.NB, gsz):
                blocks = list(range(s, min(s + gsz, self.NB)))
                chunks = []
                for b in blocks:
                    chunks += [(b, "lo")] * self.Klo[b]
                lo_n = len(chunks)
                for b in blocks:
                    chunks += [(b, "hi")] * self.Khi[b]
                groups.append({"blocks": blocks, "chunks": chunks, "lo_n": lo_n})
            return groups

        self.groups1 = make_groups(l1_group_blocks)
        self.groups2 = make_groups(l2_group_blocks)

        # global chunk numbering (shared by L1/L2: same edge stream)
        t = 0
        for g in self.groups1:
            g["t0"] = t
            t += len(g["chunks"])
        self.NCH = t
        # L2 groups reference the same chunk stream; compute their t-offsets
        # by walking blocks in the same global order. Since both group splits
        # cover blocks in order and chunks are keyed (block, kind), we build a
        # map (block,kind,i) -> t from groups1 ordering.
        self.chunk_id = {}
        for g in self.groups1:
            cnt = {}
            for i, (b, kind) in enumerate(g["chunks"]):
                k = (b, kind)
                j = cnt.get(k, 0)
                cnt[k] = j + 1
                self.chunk_id[(b, kind, j)] = g["t0"] + i

        # explicit chunk -> global column for every group (both splits)
        for gs in (self.groups1, self.groups2):
            for g in gs:
                cnt = {}
                tl = []
                for (b, kind) in g["chunks"]:
                    j = cnt.get((b, kind), 0)
                    cnt[(b, kind)] = j + 1
                    tl.append(self.chunk_id[(b, kind, j)])
                g["tlist"] = tl

        self.TOT_IDX = self.NCH * 128
        self.TOT16 = self.TOT_IDX // 16

    def pack_core(self, c, edges_src, edges_dst):
        """Build per-core gidx (int16, 16-wrapped), dstloc (bf16) arrays."""
        gidx = np.zeros((128, self.TOT16), dtype=np.int16)
        dstloc = np.full((128, self.NCH), -1.0, dtype=np.float32)
        per_block = self.edges[c]
        ns = self.core_start[c]
        for b in range(self.NB):
            lo_e, hi_e = per_block[b]
            for kind, e_list, base in (("lo", lo_e, 0), ("hi", hi_e, self.HI_BASE)):
                K = self.Klo[b] if kind == "lo" else self.Khi[b]
                for j in range(K):
                    t = self.chunk_id[(b, kind, j)]
                    seg = e_list[j * 128:(j + 1) * 128]
                    n = len(seg)
                    idxs = np.zeros(128, dtype=np.int16)
                    if n:
                        idxs[:n] = (self.prow[edges_src[seg]] - base).astype(np.int16)
                        dstloc[:n, t] = (edges_dst[seg] - ns - 128 * b).astype(np.float32)
                    # wrap: idx i -> (i%16, i//16), columns t*8 .. t*8+8;
                    # replicated to all 8 Q7 gpsimd cores (16 partitions each)
                    gidx[:, t * 8:(t + 1) * 8] = np.tile(idxs.reshape(8, 16).T,
                                                         (8, 1))
        return gidx, dstloc

    def pack_batchloc(self, c, batch):
        """Per-node local graph id (bf16), -1 for pad slots."""
        out = np.full(self.NLOC, -1.0, dtype=np.float32)
        ns, ne = self.core_start[c], self.core_end[c]
        out[: ne - ns] = batch[ns:ne] - self.g_per_core * c
        return np.ascontiguousarray(out.reshape(self.NB, 128).T)  # [128, NB]


def build_program(lay: Layout, n_cores):
    nc = bacc.Bacc(None, num_devices=n_cores)
    NLOC, NB, PROWS, NCH = lay.NLOC, lay.NB, lay.PROWS, lay.NCH
    NGL = lay.g_per_core  # graphs per core (pool output rows)
    NGP = int(math.ceil(NGL / 32) * 32)  # padded for iota tile
    replica = [list(range(n_cores))]

    with tile.TileContext(nc) as tc:
        def T(*a, **k):
            t, _free = tc.tile(*a, **k)
            return t

        res_ctx = tc.tile_pool(name="resident", bufs=1)
        res = res_ctx.__enter__()
        resp_ctx = tc.tile_pool(name="resident_ps", bufs=1, space="PSUM")
        resp = resp_ctx.__enter__()

        def R(shape, dtype, name):
            return res.tile(shape, dtype, name=name, tag=name)

        with tc.tile_pool(name="dram", bufs=1, space="DRAM") as dram:
            xloc_d = dram.tile([NLOC, NODE_DIM], F32, kind="ExternalInput", name="xloc", uniquify=False)
            wasd1_d = dram.tile([NODE_DIM, 2 * HEADS], BF16, kind="ExternalInput", name="wasd1", uniquify=False)
            w1h_d = dram.tile([NODE_DIM, HEADS * HIDDEN], BF16, kind="ExternalInput", name="w1h", uniquify=False)
            w2e_d = dram.tile([HEADS * HIDDEN, HIDDEN + 2], BF16, kind="ExternalInput", name="w2e", uniquify=False)
            fcw_d = dram.tile([HIDDEN, OUT_DIM], BF16, kind="ExternalInput", name="fcw", uniquify=False)
            iota_d = dram.tile([128, 128], BF16, kind="ExternalInput", name="iota128", uniquify=False)
            iotag_d = dram.tile([128, NGP], BF16, kind="ExternalInput", name="iotag", uniquify=False)
            idf_d = dram.tile([128, 128], F32, kind="ExternalInput", name="identf", uniquify=False)
            idb_d = dram.tile([128, 128], BF16, kind="ExternalInput", name="identb", uniquify=False)
            gidx_d = dram.tile([128, lay.TOT16], I16, kind="ExternalInput", name="gidx", uniquify=False)
            dstloc_d = dram.tile([128, NCH], F32, kind="ExternalInput", name="dstloc", uniquify=False)
            bloc_d = dram.tile([128, NB], F32, kind="ExternalInput", name="batchloc", uniquify=False)
            out_d = dram.tile([NGL, OUT_DIM], F32, kind="ExternalOutput", name="out", uniquify=False)

            xext_loc = dram.tile([NLOC, XW], BF16, name="xext_loc")
            xext_full = dram.tile([PROWS, XW], BF16, name="xext_full", addr_space="Shared")
            h2in_dram = dram.tile([NLOC, HEADS * HIDDEN], BF16, name="h2in_dram")
            h2e_loc = dram.tile([NLOC, HW2], BF16, name="h2e_loc")
            h2e_full = dram.tile([PROWS, HW2], BF16, name="h2e_full", addr_space="Shared")

        # ------------------------------------------------------------------
        # Resident SBUF tiles
        wasd1 = R([NODE_DIM, 2 * HEADS], BF16, "wasd1_sb")
        nc.sync.dma_start(out=wasd1[:], in_=wasd1_d[:])
        w1h = R([NODE_DIM, HEADS * HIDDEN], BF16, "w1h_sb")
        nc.sync.dma_start(out=w1h[:], in_=w1h_d[:])
        w2e = R([128, 8, HIDDEN + 2], BF16, "w2e_sb")
        for k in range(8):
            nc.sync.dma_start(out=w2e[:, k, :], in_=w2e_d[128 * k:128 * (k + 1), :])
        fcw = R([128, 2, OUT_DIM], BF16, "fcw_sb")
        for k in range(2):
            nc.sync.dma_start(out=fcw[:, k, :], in_=fcw_d[128 * k:128 * (k + 1), :])
        iota = R([128, 128], BF16, "iota_sb")
        nc.sync.dma_start(out=iota[:], in_=iota_d[:])
        iotag = R([128, NGP], BF16, "iotag_sb")
        nc.sync.dma_start(out=iotag[:], in_=iotag_d[:])
        identf = R([128, 128], F32, "identf_sb")
        nc.sync.dma_start(out=identf[:], in_=idf_d[:])
        identb = R([128, 128], BF16, "identb_sb")
        nc.sync.dma_start(out=identb[:], in_=idb_d[:])
        gidx = R([128, lay.TOT16], I16, "gidx_sb")
        nc.sync.dma_start(out=gidx[:], in_=gidx_d[:])
        dstloc = R([128, NCH], F32, "dstloc_sb")
        nc.sync.dma_start(out=dstloc[:], in_=dstloc_d[:])
        bloc = R([128, NB], F32, "bloc_sb")
        nc.sync.dma_start(out=bloc[:], in_=bloc_d[:])
        aldloc = R([128, NB, HEADS], BF16, "aldloc_sb")
        ald2loc = R([128, NB], BF16, "ald2loc_sb")

        # ------------------------------------------------------------------
        # Phase 1: build xext_loc ( [x | 1 | al_s] per local node )
        with tc.tile_pool(name="p1_sb", bufs=3) as p1s, \
             tc.tile_pool(name="p1_ps", bufs=2, space="PSUM") as p1p, \
             tc.tile_pool(name="p1_ps2", bufs=2, space="PSUM") as p1p2:
            for k in range(NB):
                xc = p1s.tile([128, NODE_DIM], F32, tag="xc")
                nc.sync.dma_start(out=xc[:], in_=xloc_d[128 * k:128 * (k + 1), :])
                xTp = p1p.tile([NODE_DIM, 128], F32, tag="xTp")
                nc.tensor.transpose(out=xTp[:], in_=xc[:], identity=identf[:])
                xT = p1s.tile([NODE_DIM, 128], BF16, tag="xT")
                nc.scalar.activation(out=xT[:], in_=xTp[:],
                                     func=mybir.ActivationFunctionType.Copy)
                alp = p1p2.tile([128, 2 * HEADS], F32, tag="alp")
                nc.tensor.matmul(out=alp[:], lhsT=xT[:], rhs=wasd1[:],
                                 start=True, stop=True)
                xe = p1s.tile([128, XW], BF16, tag="xe")
                nc.vector.tensor_copy(out=xe[:, 0:NODE_DIM], in_=xc[:])
                nc.vector.memset(xe[:, COL_ONE:COL_ONE + 1], 1.0)
                nc.vector.tensor_copy(out=xe[:, COL_ALS:COL_ALS + HEADS],
                                      in_=alp[:, 0:HEADS])
                nc.vector.memset(xe[:, COL_ALS + HEADS:XW], 0.0)
                nc.vector.tensor_copy(out=aldloc[:, k, :], in_=alp[:, HEADS:2 * HEADS])
                nc.sync.dma_start(out=xext_loc[128 * k:128 * (k + 1), :], in_=xe[:])

        nc.gpsimd.collective_compute(
            "AllGather", mybir.AluOpType.bypass, replica_groups=replica,
            ins=[xext_loc[:]], outs=[xext_full[:]])

        # ------------------------------------------------------------------
        # Layer helpers
        def gat_layer(groups, table_full, elem_w, agg_w, col_als, n_heads,
                      hi_base, post_block):
            """Shared L1/L2 edge-processing machinery."""
            with tc.tile_pool(name="g_sb", bufs=2) as gsb, \
                 tc.tile_pool(name="mt_sb", bufs=10) as msb, \
                 tc.tile_pool(name="sc_sb", bufs=4) as ssb, \
                 tc.tile_pool(name="xs_sb", bufs=4) as xsb, \
                 tc.tile_pool(name="ag_ps", bufs=2, space="PSUM") as agp, \
                 tc.tile_pool(name="mt_ps", bufs=2, space="PSUM") as mtp, \
                 tc.tile_pool(name="ad_ps", bufs=1, space="PSUM") as adp, \
                 tc.tile_pool(name="po_ps", bufs=2, space="PSUM") as pop:
                for g in groups:
                    nch = len(g["chunks"])
                    gt = gsb.tile([128, nch, elem_w], BF16, tag="gt")
                    tl = g["tlist"]
                    # contiguous (kind, t) runs -> one dma_gather each,
                    # capped at MAX_GATHER_CHUNKS per call (huge descriptor
                    # counts in one SWDGE call hang the device)
                    r0 = 0
                    while r0 < nch:
                        r1 = r0 + 1
                        while (r1 < nch and r1 - r0 < MAX_GATHER_CHUNKS
                               and tl[r1] == tl[r1 - 1] + 1
                               and g["chunks"][r1][1] == g["chunks"][r0][1]):
                            r1 += 1
                        kind = g["chunks"][r0][1]
                        base = 0 if kind == "lo" else hi_base
                        n = (r1 - r0) * 128
                        nc.gpsimd.dma_gather(
                            out_ap=gt[:, r0:r1, :],
                            in_ap=table_full[base:, :],
                            idxs_ap=gidx[:, tl[r0] * 8:(tl[r1 - 1] + 1) * 8],
                            num_idxs=n, num_idxs_reg=n,
                            elem_size=elem_w)
                        r0 = r1
                    # chunk index within this group per block
                    by_block = {}
                    for i, (b, kind) in enumerate(g["chunks"]):
                        by_block.setdefault(b, []).append(i)
                    for b in g["blocks"]:
                        idxs = by_block.get(b, [])
                        ncb = len(idxs)
                        if ncb == 0:
                            continue
                        mts = []
                        aldp = adp.tile([128, ncb, n_heads], F32, tag="aldp")
                        for j, i in enumerate(idxs):
                            t = tl[i]
                            mt = msb.tile([128, 128], BF16, tag="mt")
                            nc.vector.tensor_scalar(
                                out=mt[:], in0=iota[:],
                                scalar1=dstloc[:, t:t + 1], scalar2=None,
                                op0=mybir.AluOpType.is_equal)
                            mts.append(mt)
                            mtt = mtp.tile([128, 128], BF16, tag="mtt")
                            nc.tensor.transpose(out=mtt[:], in_=mt[:],
                                                identity=identb[:])
                            mn = msb.tile([128, 128], BF16, tag="mn")
                            nc.scalar.activation(out=mn[:], in_=mtt[:],
                                                 func=mybir.ActivationFunctionType.Copy)
                            if n_heads > 1:
                                rhs_ald = aldloc[:, b, :]
                            else:
                                rhs_ald = ald2loc[:, b:b + 1]
                            nc.tensor.matmul(out=aldp[:, j, :], lhsT=mn[:],
                                             rhs=rhs_ald, start=True, stop=True)
                        # logits for the whole block: z = al_s(gather) + al_d.
                        # A block's chunks form <=2 contiguous runs in the
                        # group supertile (its lo chunks, then its hi chunks).
                        z = ssb.tile([128, ncb, n_heads], F32, tag="z")
                        s0 = 0
                        while s0 < ncb:
                            s1 = s0 + 1
                            while s1 < ncb and idxs[s1] == idxs[s1 - 1] + 1:
                                s1 += 1
                            als_view = gt[:, idxs[s0]:idxs[s0] + (s1 - s0),
                                          col_als:col_als + n_heads]
                            nc.vector.tensor_tensor(
                                out=z[:, s0:s1, :], in0=als_view,
                                in1=aldp[:, s0:s1, :], op=mybir.AluOpType.add)
                            s0 = s1
                        e1 = ssb.tile([128, ncb, n_heads], F32, tag="e1")
                        nc.scalar.activation(out=e1[:], in_=z[:],
                                             func=mybir.ActivationFunctionType.Exp)
                        e2 = ssb.tile([128, ncb, n_heads], F32, tag="e2")
                        nc.scalar.activation(out=e2[:], in_=z[:],
                                             func=mybir.ActivationFunctionType.Exp,
                                             scale=float(NEG_SLOPE))
                        ah = ssb.tile([128, ncb, n_heads], F32, tag="ah")
                        nc.vector.tensor_tensor(out=ah[:], in0=e1[:], in1=e2[:],
                                                op=mybir.AluOpType.max)
                        # aggregation: one matmul per chunk, all heads side
                        # by side in the rhs (single PSUM accumulation group)
                        aggp = agp.tile([128, n_heads, agg_w], F32, tag="aggp")
                        for j, i in enumerate(idxs):
                            xs = xsb.tile([128, n_heads, agg_w], BF16, tag="xs")
                            for h in range(n_heads):
                                nc.vector.tensor_scalar(
                                    out=xs[:, h, :], in0=gt[:, i, 0:agg_w],
                                    scalar1=ah[:, j, h:h + 1], scalar2=None,
                                    op0=mybir.AluOpType.mult)
                            nc.tensor.matmul(out=aggp[:], lhsT=mts[j][:],
                                             rhs=xs[:], start=(j == 0),
                                             stop=(j == ncb - 1))
                        post_block(b, aggp)

        # ------------------------------------------------------------------
        # Phase 2: layer 1
        with tc.tile_pool(name="b1_sb", bufs=3) as b1s, \
             tc.tile_pool(name="b1h_sb", bufs=2) as b1h, \
             tc.tile_pool(name="b1_ps", bufs=1, space="PSUM") as b1p, \
             tc.tile_pool(name="w1_ps", bufs=1, space="PSUM") as w1p:

            def post1(b, aggp):
                hb = b1h.tile([128, HEADS * HIDDEN], BF16, tag="hb")
                for h in range(HEADS):
                    den = b1s.tile([128, 1], F32, tag="den")
                    nc.vector.tensor_scalar(
                        out=den[:], in0=aggp[:, h, NODE_DIM:NODE_DIM + 1],
                        scalar1=1e-30, scalar2=None, op0=mybir.AluOpType.max)
                    rec = b1s.tile([128, 1], F32, tag="rec")
                    nc.vector.reciprocal(out=rec[:], in_=den[:])
                    axn = b1s.tile([128, NODE_DIM], BF16, tag="axn")
                    nc.vector.tensor_scalar(
                        out=axn[:], in0=aggp[:, h, 0:NODE_DIM], scalar1=rec[:],
                        scalar2=None, op0=mybir.AluOpType.mult)
                    axTp = b1p.tile([NODE_DIM, 128], BF16, tag="axTp")
                    nc.tensor.transpose(out=axTp[:], in_=axn[:], identity=identb[:])
                    axT = b1s.tile([NODE_DIM, 128], BF16, tag="axT")
                    nc.scalar.activation(out=axT[:], in_=axTp[:],
                                         func=mybir.ActivationFunctionType.Copy)
                    h1p = w1p.tile([128, HIDDEN], F32, tag="h1p")
                    nc.tensor.matmul(out=h1p[:], lhsT=axT[:],
                                     rhs=w1h[:, HIDDEN * h:HIDDEN * (h + 1)],
                                     start=True, stop=True)
                    nc.scalar.activation(out=hb[:, HIDDEN * h:HIDDEN * (h + 1)],
                                         in_=h1p[:],
                                         func=mybir.ActivationFunctionType.Relu)
                nc.sync.dma_start(out=h2in_dram[128 * b:128 * (b + 1), :], in_=hb[:])

            gat_layer(lay.groups1, xext_full, XW, AGG_W, COL_ALS, HEADS,
                      lay.HI_BASE, post1)

        # ------------------------------------------------------------------
        # Phase 3: h2 = relu(h1) @ W2ext  (also yields al_s2, al_d2)
        with tc.tile_pool(name="p3_sb", bufs=3) as p3s, \
             tc.tile_pool(name="p3_ps", bufs=2, space="PSUM") as p3p:
            for j in range(NB):
                h2p = p3p.tile([128, HIDDEN + 2], F32, tag="h2p")
                for k in range(8):
                    hT = p3s.tile([128, 128], BF16, tag="hT")
                    nc.sync.dma_start(
                        out=hT[:],
                        in_=h2in_dram[128 * j:128 * (j + 1), 128 * k:128 * (k + 1)],
                        transpose=True)
                    nc.tensor.matmul(out=h2p[:], lhsT=hT[:], rhs=w2e[:, k, :],
                                     start=(k == 0), stop=(k == 7))
                he = p3s.tile([128, HW2], BF16, tag="he")
                nc.scalar.activation(out=he[:, 0:HIDDEN], in_=h2p[:, 0:HIDDEN],
                                     func=mybir.ActivationFunctionType.Copy)
                nc.vector.memset(he[:, H_COL_ONE:H_COL_ONE + 1], 1.0)
                nc.vector.tensor_copy(out=he[:, H_COL_ALS:H_COL_ALS + 1],
                                      in_=h2p[:, HIDDEN:HIDDEN + 1])
                nc.vector.memset(he[:, H_COL_ALS + 1:HW2], 0.0)
                nc.vector.tensor_copy(out=ald2loc[:, j:j + 1],
                                      in_=h2p[:, HIDDEN + 1:HIDDEN + 2])
                nc.sync.dma_start(out=h2e_loc[128 * j:128 * (j + 1), :], in_=he[:])

        nc.gpsimd.collective_compute(
            "AllGather", mybir.AluOpType.bypass, replica_groups=replica,
            ins=[h2e_loc[:]], outs=[h2e_full[:]])

        # ------------------------------------------------------------------
        # Phase 4: layer 2 + pooling accumulation
        poolp = resp.tile([NGL, HIDDEN + 1], F32, name="poolp", tag="poolp")
        n_fire = sum(1 for b in range(NB) if lay.Klo[b] + lay.Khi[b] > 0)
        nblk_done = [0]
        with tc.tile_pool(name="b2_sb", bufs=4) as b2s:

            def post2(b, aggp):
                den = b2s.tile([128, 1], F32, tag="den2")
                nc.vector.tensor_scalar(
                    out=den[:], in0=aggp[:, 0, HIDDEN:HIDDEN + 1],
                    scalar1=1e-30, scalar2=None, op0=mybir.AluOpType.max)
                rec = b2s.tile([128, 1], F32, tag="rec2")
                nc.vector.reciprocal(out=rec[:], in_=den[:])
                hf = b2s.tile([128, HIDDEN + 1], BF16, tag="hf")
                nc.vector.tensor_scalar(
                    out=hf[:, 0:HIDDEN], in0=aggp[:, 0, 0:HIDDEN],
                    scalar1=rec[:], scalar2=0.0,
                    op0=mybir.AluOpType.mult, op1=mybir.AluOpType.max)
                nc.vector.memset(hf[:, HIDDEN:HIDDEN + 1], 1.0)
                mg = b2s.tile([128, NGP], BF16, tag="mg")
                nc.vector.tensor_scalar(
                    out=mg[:], in0=iotag[:], scalar1=bloc[:, b:b + 1],
                    scalar2=None, op0=mybir.AluOpType.is_equal)
                nc.tensor.matmul(out=poolp[:], lhsT=mg[:, 0:NGL], rhs=hf[:],
                                 start=(nblk_done[0] == 0),
                                 stop=(nblk_done[0] == n_fire - 1))
                nblk_done[0] += 1

            gat_layer(lay.groups2, h2e_full, HW2, H_AGG_W, H_COL_ALS, 1,
                      lay.HI_BASE, post2)

        # ------------------------------------------------------------------
        # Phase 5: mean + FC
        with tc.tile_pool(name="p5_sb", bufs=2) as p5s, \
             tc.tile_pool(name="p5_ps", bufs=2, space="PSUM") as p5p:
            cnt = p5s.tile([NGL, 1], F32, name="cnt")
            nc.vector.tensor_scalar(out=cnt[:], in0=poolp[:, HIDDEN:HIDDEN + 1],
                                    scalar1=1.0, scalar2=None,
                                    op0=mybir.AluOpType.max)
            crec = p5s.tile([NGL, 1], F32, name="crec")
            nc.vector.reciprocal(out=crec[:], in_=cnt[:])
            pm = p5s.tile([NGL, HIDDEN], BF16, name="pm")
            nc.vector.tensor_scalar(out=pm[:], in0=poolp[:, 0:HIDDEN],
                                    scalar1=crec[:], scalar2=None,
                                    op0=mybir.AluOpType.mult)
            fcp = p5p.tile([NGL, OUT_DIM], F32, name="fcp")
            for k in range(2):
                pmTp = p5p.tile([128, NGL], BF16, tag="pmTp")
                nc.tensor.transpose(out=pmTp[:], in_=pm[:, 128 * k:128 * (k + 1)],
                                    identity=identb[0:NGL, 0:NGL])
                pmT = p5s.tile([128, NGL], BF16, tag="pmT")
                nc.scalar.activation(out=pmT[:], in_=pmTp[:],
                                     func=mybir.ActivationFunctionType.Copy)
                nc.tensor.matmul(out=fcp[:], lhsT=pmT[:], rhs=fcw[:, k, :],
                                 start=(k == 0), stop=(k == 1))
            outs = p5s.tile([NGL, OUT_DIM], F32, name="outs")
            nc.vector.tensor_copy(out=outs[:], in_=fcp[:])
            nc.sync.dma_start(out=out_d[:], in_=outs[:])

        resp_ctx.__exit__(None, None, None)
        res_ctx.__exit__(None, None, None)

    nc.compile()
    return nc


def prep_inputs(x, edge_index, batch, W1, a_src1, a_dst1, b1, W2, a_src2,
                a_dst2, b2, fc_W, fc_b, n_cores=N_CORES,
                l1_group_blocks=16, l2_group_blocks=8):
    """Host-side: shard + pack all per-core input tensors."""
    n = x.shape[0]
    src = np.concatenate([np.asarray(edge_index[0]), np.arange(n)]).astype(np.int64)
    dst = np.concatenate([np.asarray(edge_index[1]), np.arange(n)]).astype(np.int64)
    batch = np.asarray(batch).astype(np.int64)
    x = np.asarray(x, dtype=np.float32)

    lay = Layout(n, int(batch.max()) + 1, n_cores, src, dst, batch,
                 l1_group_blocks=l1_group_blocks,
                 l2_group_blocks=l2_group_blocks)

    bf = ml_dtypes.bfloat16
    W1 = np.asarray(W1, np.float32)
    was1 = np.einsum("dhk,hk->dh", W1.reshape(NODE_DIM, HEADS, HIDDEN),
                     np.asarray(a_src1, np.float32))
    wad1 = np.einsum("dhk,hk->dh", W1.reshape(NODE_DIM, HEADS, HIDDEN),
                     np.asarray(a_dst1, np.float32))
    wasd1 = np.concatenate([was1, wad1], axis=1).astype(bf)
    W2 = np.asarray(W2, np.float32)
    w2e = np.concatenate([
        W2,
        (W2 @ np.asarray(a_src2, np.float32)[0])[:, None],
        (W2 @ np.asarray(a_dst2, np.float32)[0])[:, None],
    ], axis=1).astype(bf)
    NGP = int(math.ceil(lay.g_per_core / 32) * 32)
    iota128 = np.tile(np.arange(128, dtype=np.float32), (128, 1)).astype(bf)
    iotag = np.tile(np.arange(NGP, dtype=np.float32), (128, 1)).astype(bf)
    identf = np.eye(128, dtype=np.float32)
    identb = np.eye(128, dtype=np.float32).astype(bf)

    common = {
        "wasd1": wasd1,
        "w1h": W1.astype(bf),
        "w2e": w2e,
        "fcw": np.asarray(fc_W, np.float32).astype(bf),
        "iota128": iota128,
        "iotag": iotag,
        "identf": identf,
        "identb": identb,
    }
    in_maps = []
    for c in range(n_cores):
        gidx, dstloc = lay.pack_core(c, src, dst)
        xl = np.zeros((lay.NLOC, NODE_DIM), dtype=np.float32)
        ns, ne = lay.core_start[c], lay.core_end[c]
        xl[: ne - ns] = x[ns:ne]
        m = dict(common)
        m["xloc"] = xl
        m["gidx"] = gidx
        m["dstloc"] = dstloc
        m["batchloc"] = lay.pack_batchloc(c, batch)
        in_maps.append(m)
    return lay, in_maps


def kernel(**inputs) -> np.ndarray:
    lay, in_maps = prep_inputs(**inputs)
    nc = build_program(lay, N_CORES)
    res = run_bass_kernel_spmd(nc, in_maps, list(range(N_CORES)))
    outs = [np.asarray(res.results[c]["out"], dtype=np.float32)
            for c in range(N_CORES)]
    return np.concatenate(outs, axis=0)



# revision 9
# speedup vs baseline: 1.6757x; 1.6757x over previous
"""Trainium2 Bass kernel for a 2-layer GAT + global-mean-pool + FC model.

Strategy (8 NeuronCores, SPMD):
  - Nodes partitioned across cores at graph boundaries (32 graphs/core),
    padded to NLOC rows per core; "padded row id" space is the concatenation
    of all cores' padded segments (PROWS rows total).
  - GAT aggregation is linear in the source features, so layer 1 aggregates
    the 74-dim inputs (not 1024-dim hidden):  out1[dst,h] =
    (sum_e alpha_eh * x[src_e]) @ W1_h   -- cuts edge-gather traffic ~14x.
  - Self-loop edges (1 per node, 20% of the edge stream) are NOT gathered;
    their alpha*x contribution is folded in locally per 128-node block.
  - Each core builds per-node tables ([x|1|al_s] for L1, h2 for L2),
    AllGathers them, then processes edges whose dst it owns in groups of
    blocks, with per-group FUSED vector ops (broadcast/stride-0 APs):
      * one-hot edge->dst matrices built two ways in one DVE op each:
        mts[e,d] (agg lhsT) from iota vs dstloc, mns[d,e] (al_dst lhsT)
        from a host-shipped replicated dst row,
      * attention: z = al_src(gathered) + (mns @ al_dst);
        exp(leakyrelu(z)) = max(e^z, e^.2z); alpha-scaled rhs built with one
        broadcast tensor_tensor per head per group,
      * per 128-edge chunk: one matmul accumulates [sum alpha*x | sum alpha]
        into PSUM per dst block.
  - Layer 2 gathers 256-col (512B) h2 rows only; per-edge al_src2 is an
    on-chip dot with a_src2 (tensor_tensor_reduce), and the softmax
    denominator rides as an extra alpha column in the matmul rhs.
  - h1 is produced TRANSPOSED ([hid,node] slices) straight out of layer-1
    PSUM, so h2 = relu(h1)@W2 needs no DMA transposes / DRAM round trip.
  - Pooling: one-hot (node->graph) matmuls into a resident PSUM tile; ones
    column yields node counts. Final FC on-chip, [32,12] per core.

All per-core variation travels through input tensors (SPMD: one program).
"""

import math
import sys

sys.path.insert(0, "/opt/trn_rl_repo")

import numpy as np
import ml_dtypes

import concourse.bass as bass
import concourse.bacc as bacc
import concourse.mybir as mybir
import concourse.tile as tile
from concourse.bass_utils import run_bass_kernel_spmd

BF16 = mybir.dt.bfloat16
F32 = mybir.dt.float32
I16 = mybir.dt.int16

NEG_SLOPE = 0.2

# ---------------------------------------------------------------------------
# Model dims (problem constants)
N_NODES = 50000
N_EDGES = 200000
NODE_DIM = 74
HIDDEN = 256
HEADS = 4
OUT_DIM = 12
N_GRAPHS = 256
N_CORES = 8

# xext row: [x(74) | 1.0 | al_s(HEADS) | pad] in bf16, padded to XW cols
XW = 128
COL_ONE = NODE_DIM          # 74
COL_ALS = NODE_DIM + 1      # 75
AGG_W = NODE_DIM + 1        # 75: matmul rhs slice [x | 1]

# h2 table row: plain h2 (256 cols bf16 = 512B, the gather granularity floor)
HW2 = HIDDEN                # 256

LO_LIMIT = 28672  # int16 gather index limit (values near 32767 fault the ucode)
MAX_GATHER_CHUNKS = 8  # cap descriptors per dma_gather call


class Layout:
    """Static (core-uniform) layout computed on the host from the edge data.

    Self-loops are excluded from the edge stream (handled locally).
    L1 and L2 share one group split so every group covers a contiguous
    global chunk range [t0, t0+nch).
    """

    def __init__(self, n_nodes, n_graphs, n_cores, edges_src, edges_dst, batch,
                 group_blocks=8):
        self.n_cores = n_cores
        g_per_core = n_graphs // n_cores
        assert g_per_core * n_cores == n_graphs
        gb = np.searchsorted(batch, np.arange(n_graphs + 1))
        self.core_start = gb[np.arange(n_cores) * g_per_core]
        self.core_end = gb[(np.arange(n_cores) + 1) * g_per_core]
        n_local = self.core_end - self.core_start
        self.NLOC = int(math.ceil(n_local.max() / 128) * 128)
        self.NB = self.NLOC // 128
        self.PROWS = self.NLOC * n_cores
        assert self.PROWS <= 2 * LO_LIMIT, "lo/hi gather windows must cover all rows"
        self.HI_BASE = self.PROWS - LO_LIMIT if self.PROWS > LO_LIMIT else 0
        self.g_per_core = g_per_core

        # node -> (core, padded row)
        core_of = np.searchsorted(self.core_end, np.arange(n_nodes), side="right")
        prow = self.NLOC * core_of + (np.arange(n_nodes) - self.core_start[core_of])
        self.prow = prow

        dst_core = core_of[edges_dst]
        dstloc_all = edges_dst - self.core_start[dst_core]
        blk = dstloc_all // 128
        src_p = prow[edges_src]
        is_lo = src_p < LO_LIMIT

        self.edges = []  # per core: dict(block -> (lo_idx_array, hi_idx_array))
        nlo = np.zeros((n_cores, self.NB), dtype=np.int64)
        nhi = np.zeros((n_cores, self.NB), dtype=np.int64)
        for c in range(n_cores):
            sel = np.nonzero(dst_core == c)[0]
            per_block = {}
            bsel = blk[sel]
            for b in range(self.NB):
                e_b = sel[bsel == b]
                lo_e = e_b[is_lo[e_b]]
                hi_e = e_b[~is_lo[e_b]]
                per_block[b] = (lo_e, hi_e)
                nlo[c, b] = len(lo_e)
                nhi[c, b] = len(hi_e)
            self.edges.append(per_block)

        self.Klo = np.ceil(nlo.max(axis=0) / 128).astype(int)
        self.Khi = np.ceil(nhi.max(axis=0) / 128).astype(int)

        # groups of blocks; chunk order within a group: all lo chunks (by
        # block), then all hi chunks (by block). Global chunk ids are
        # assigned in group order, so every group covers [t0, t0+nch).
        self.groups = []
        for s in range(0, self.NB, group_blocks):
            blocks = list(range(s, min(s + group_blocks, self.NB)))
            chunks = []
            for b in blocks:
                chunks += [(b, "lo")] * self.Klo[b]
            lo_n = len(chunks)
            for b in blocks:
                chunks += [(b, "hi")] * self.Khi[b]
            self.groups.append({"blocks": blocks, "chunks": chunks, "lo_n": lo_n})
        t = 0
        for g in self.groups:
            g["t0"] = t
            t += len(g["chunks"])
        self.NCH = max(t, 1)
        self.chunk_id = {}
        for g in self.groups:
            cnt = {}
            for i, (b, kind) in enumerate(g["chunks"]):
                k = (b, kind)
                j = cnt.get(k, 0)
                cnt[k] = j + 1
                self.chunk_id[(b, kind, j)] = g["t0"] + i

        self.TOT_IDX = self.NCH * 128
        self.TOT16 = self.TOT_IDX // 16

    def pack_core(self, c, edges_src, edges_dst):
        """Per-core gidx (int16, 16-wrapped), dstloc [128,NCH], dstrep [1,NCH*128]."""
        gidx = np.zeros((128, self.TOT16), dtype=np.int16)
        dstloc = np.full((128, self.NCH), -1.0, dtype=np.float32)
        per_block = self.edges[c]
        ns = self.core_start[c]
        for b in range(self.NB):
            lo_e, hi_e = per_block[b]
            for kind, e_list, base in (("lo", lo_e, 0), ("hi", hi_e, self.HI_BASE)):
                K = self.Klo[b] if kind == "lo" else self.Khi[b]
                for j in range(K):
                    t = self.chunk_id[(b, kind, j)]
                    seg = e_list[j * 128:(j + 1) * 128]
                    n = len(seg)
                    idxs = np.zeros(128, dtype=np.int16)
                    if n:
                        idxs[:n] = (self.prow[edges_src[seg]] - base).astype(np.int16)
                        dstloc[:n, t] = (edges_dst[seg] - ns - 128 * b).astype(np.float32)
                    # wrap: idx i -> (i%16, i//16), columns t*8 .. t*8+8;
                    # replicated to all 8 Q7 gpsimd cores (16 partitions each)
                    gidx[:, t * 8:(t + 1) * 8] = np.tile(idxs.reshape(8, 16).T,
                                                         (8, 1))
        bf = ml_dtypes.bfloat16
        dstrep = np.ascontiguousarray(dstloc.T.reshape(1, -1)).astype(bf)
        return gidx, dstloc.astype(bf), dstrep

    def pack_batchloc(self, c, batch):
        """Per-node local graph id, -1 for pad slots -> [128, NB] bf16."""
        out = np.full(self.NLOC, -1.0, dtype=np.float32)
        ns, ne = self.core_start[c], self.core_end[c]
        out[: ne - ns] = batch[ns:ne] - self.g_per_core * c
        return np.ascontiguousarray(out.reshape(self.NB, 128).T).astype(
            ml_dtypes.bfloat16)


def build_program(lay: Layout, n_cores):
    nc = bacc.Bacc(None, num_devices=n_cores)
    NLOC, NB, PROWS, NCH = lay.NLOC, lay.NB, lay.PROWS, lay.NCH
    NGL = lay.g_per_core  # graphs per core (pool output rows)
    NGP = int(math.ceil(NGL / 32) * 32)  # padded for iota tile
    replica = [list(range(n_cores))]
    H = HEADS

    with tile.TileContext(nc) as tc:
        res_ctx = tc.tile_pool(name="resident", bufs=1)
        res = res_ctx.__enter__()

        def R(shape, dtype, name):
            return res.tile(shape, dtype, name=name, tag=name)

        with tc.tile_pool(name="dram", bufs=1, space="DRAM") as dram:
            xloc_d = dram.tile([NLOC, NODE_DIM], BF16, kind="ExternalInput", name="xloc", uniquify=False)
            wasd1_d = dram.tile([NODE_DIM, 2 * HEADS], BF16, kind="ExternalInput", name="wasd1", uniquify=False)
            w1h_d = dram.tile([NODE_DIM, HEADS * HIDDEN], BF16, kind="ExternalInput", name="w1h", uniquify=False)
            w2e_d = dram.tile([HEADS * HIDDEN, HIDDEN + 2], BF16, kind="ExternalInput", name="w2e", uniquify=False)
            a2s_d = dram.tile([1, HIDDEN], BF16, kind="ExternalInput", name="a2srow", uniquify=False)
            fcw_d = dram.tile([HIDDEN, OUT_DIM], BF16, kind="ExternalInput", name="fcw", uniquify=False)
            iota_d = dram.tile([128, 128], BF16, kind="ExternalInput", name="iota128", uniquify=False)
            iotac_d = dram.tile([128, 1], F32, kind="ExternalInput", name="iotacol", uniquify=False)
            iotag_d = dram.tile([128, NGP], BF16, kind="ExternalInput", name="iotag", uniquify=False)
            idb_d = dram.tile([128, 128], BF16, kind="ExternalInput", name="identb", uniquify=False)
            gidx_d = dram.tile([128, lay.TOT16], I16, kind="ExternalInput", name="gidx", uniquify=False)
            dstloc_d = dram.tile([128, NCH], BF16, kind="ExternalInput", name="dstloc", uniquify=False)
            dstrep_d = dram.tile([1, NCH * 128], BF16, kind="ExternalInput", name="dstrep", uniquify=False)
            bloc_d = dram.tile([128, NB], BF16, kind="ExternalInput", name="batchloc", uniquify=False)
            out_d = dram.tile([NGL, OUT_DIM], F32, kind="ExternalOutput", name="out", uniquify=False)

            xext_loc = dram.tile([NLOC, XW], BF16, name="xext_loc")
            xext_full = dram.tile([PROWS, XW], BF16, name="xext_full", addr_space="Shared")
            h2e_loc = dram.tile([NLOC, HW2], BF16, name="h2e_loc")
            h2e_full = dram.tile([PROWS, HW2], BF16, name="h2e_full", addr_space="Shared")

        # ------------------------------------------------------------------
        # Resident SBUF tiles
        wasd1 = R([NODE_DIM, 2 * HEADS], BF16, "wasd1_sb")
        nc.sync.dma_start(out=wasd1[:], in_=wasd1_d[:])
        w1h = R([NODE_DIM, HEADS * HIDDEN], BF16, "w1h_sb")
        nc.sync.dma_start(out=w1h[:], in_=w1h_d[:])
        w2e = R([128, 8, HIDDEN + 2], BF16, "w2e_sb")
        for k in range(8):
            nc.sync.dma_start(out=w2e[:, k, :], in_=w2e_d[128 * k:128 * (k + 1), :])
        a2srep = R([128, HIDDEN], BF16, "a2srep_sb")
        nc.sync.dma_start(out=a2srep[:], in_=a2s_d[:].broadcast_to([128, HIDDEN]))
        fcw = R([128, 2, OUT_DIM], BF16, "fcw_sb")
        for k in range(2):
            nc.sync.dma_start(out=fcw[:, k, :], in_=fcw_d[128 * k:128 * (k + 1), :])
        iota = R([128, 128], BF16, "iota_sb")
        nc.sync.dma_start(out=iota[:], in_=iota_d[:])
        iotac = R([128, 1], F32, "iotac_sb")
        nc.sync.dma_start(out=iotac[:], in_=iotac_d[:])
        iotag = R([128, NGP], BF16, "iotag_sb")
        nc.sync.dma_start(out=iotag[:], in_=iotag_d[:])
        identb = R([128, 128], BF16, "identb_sb")
        nc.sync.dma_start(out=identb[:], in_=idb_d[:])
        gidx = R([128, lay.TOT16], I16, "gidx_sb")
        nc.sync.dma_start(out=gidx[:], in_=gidx_d[:])
        dstloc = R([128, NCH], BF16, "dstloc_sb")
        nc.sync.dma_start(out=dstloc[:], in_=dstloc_d[:])
        bloc = R([128, NB], BF16, "bloc_sb")
        nc.sync.dma_start(out=bloc[:], in_=bloc_d[:])

        xloc_sb = R([128, NB, NODE_DIM], BF16, "xloc_sb")
        aldloc = R([128, NB, 2 * H], BF16, "aldloc_sb")      # [al_s | al_d] L1
        al2loc = R([128, NB, 2], BF16, "al2loc_sb")          # [al_s2 | al_d2]
        h2loc_sb = R([128, NB, HIDDEN], BF16, "h2loc_sb")
        mg_all = R([128, NB, NGP], BF16, "mg_all_sb")

        # pool one-hot for every block in one fused op
        nc.vector.tensor_tensor(
            out=mg_all[:],
            in0=iotag[:].unsqueeze(1).broadcast_to([128, NB, NGP]),
            in1=bloc[:].unsqueeze(2).broadcast_to([128, NB, NGP]),
            op=mybir.AluOpType.is_equal)

        # ------------------------------------------------------------------
        # Phase 1: build xext_loc ( [x | 1 | al_s] per local node )
        with tc.tile_pool(name="p1_sb", bufs=3) as p1s, \
             tc.tile_pool(name="p1_ps", bufs=2, space="PSUM") as p1p, \
             tc.tile_pool(name="p1_ps2", bufs=2, space="PSUM") as p1p2:
            for k in range(NB):
                xc = xloc_sb[:, k, :]
                nc.sync.dma_start(out=xc, in_=xloc_d[128 * k:128 * (k + 1), :])
                xTp = p1p.tile([NODE_DIM, 128], BF16, tag="xTp")
                nc.tensor.transpose(out=xTp[:], in_=xc, identity=identb[:])
                xT = p1s.tile([NODE_DIM, 128], BF16, tag="xT")
                nc.scalar.activation(out=xT[:], in_=xTp[:],
                                     func=mybir.ActivationFunctionType.Copy)
                alp = p1p2.tile([128, 2 * HEADS], F32, tag="alp")
                nc.tensor.matmul(out=alp[:], lhsT=xT[:], rhs=wasd1[:],
                                 start=True, stop=True)
                nc.vector.tensor_copy(out=aldloc[:, k, :], in_=alp[:])
                xe = p1s.tile([128, XW], BF16, tag="xe")
                nc.vector.tensor_copy(out=xe[:, 0:NODE_DIM], in_=xc)
                nc.vector.memset(xe[:, COL_ONE:COL_ONE + 1], 1.0)
                nc.vector.tensor_copy(out=xe[:, COL_ALS:COL_ALS + HEADS],
                                      in_=alp[:, 0:HEADS])
                nc.vector.memset(xe[:, COL_ALS + HEADS:XW], 0.0)
                nc.sync.dma_start(out=xext_loc[128 * k:128 * (k + 1), :], in_=xe[:])

        nc.gpsimd.collective_compute(
            "AllGather", mybir.AluOpType.bypass, replica_groups=replica,
            ins=[xext_loc[:]], outs=[xext_full[:]])

        # ------------------------------------------------------------------
        # Layer 1: per-group fused edge processing + per-block post
        with tc.tile_pool(name="g1_sb", bufs=2) as gpool, \
             tc.tile_pool(name="d1_sb", bufs=2) as dpool, \
             tc.tile_pool(name="m1_sb", bufs=2) as mpool, \
             tc.tile_pool(name="n1_sb", bufs=2) as npool, \
             tc.tile_pool(name="x1_sb", bufs=1) as xpool, \
             tc.tile_pool(name="z1_sb", bufs=2) as zpool, \
             tc.tile_pool(name="q1_sb", bufs=3) as qpool, \
             tc.tile_pool(name="ad1_ps", bufs=2, space="PSUM") as adps, \
             tc.tile_pool(name="ag1_ps", bufs=2, space="PSUM") as agps, \
             tc.tile_pool(name="tr1_ps", bufs=1, space="PSUM") as trps, \
             tc.tile_pool(name="h1_ps", bufs=1, space="PSUM") as h1ps, \
             tc.tile_pool(name="h2_ps", bufs=1, space="PSUM") as h2ps:

            for g in lay.groups:
                nch = len(g["chunks"])
                t0 = g["t0"]
                by_block = {}
                for i, (b, kind) in enumerate(g["chunks"]):
                    by_block.setdefault(b, []).append(i)

                if nch:
                    gt = gpool.tile([128, nch, XW], BF16, tag="gt")
                    for r0, r1, base in _gather_runs(g, lay):
                        n = (r1 - r0) * 128
                        nc.gpsimd.dma_gather(
                            out_ap=gt[:, r0:r1, :],
                            in_ap=xext_full[base:, :],
                            idxs_ap=gidx[:, (t0 + r0) * 8:(t0 + r1) * 8],
                            num_idxs=n, num_idxs_reg=n, elem_size=XW)
                    dsr = dpool.tile([128, nch, 128], BF16, tag="dsr")
                    nc.sync.dma_start(
                        out=dsr[:],
                        in_=dstrep_d[:, t0 * 128:(t0 + nch) * 128]
                        .rearrange("a (b c) -> a b c", c=128)
                        .broadcast_to([128, nch, 128]))
                    mts = mpool.tile([128, nch, 128], BF16, tag="mts")
                    nc.vector.tensor_tensor(
                        out=mts[:],
                        in0=iota[:].unsqueeze(1).broadcast_to([128, nch, 128]),
                        in1=dstloc[:, t0:t0 + nch].unsqueeze(2)
                        .broadcast_to([128, nch, 128]),
                        op=mybir.AluOpType.is_equal)
                    mns = npool.tile([128, nch, 128], BF16, tag="mns")
                    nc.vector.tensor_scalar(
                        out=mns[:], in0=dsr[:], scalar1=iotac[:], scalar2=None,
                        op0=mybir.AluOpType.is_equal)
                    aldp = adps.tile([128, nch, H], F32, tag="aldp")
                    for i, (b, kind) in enumerate(g["chunks"]):
                        nc.tensor.matmul(out=aldp[:, i, :], lhsT=mns[:, i, :],
                                         rhs=aldloc[:, b, H:2 * H],
                                         start=True, stop=True)
                    z = zpool.tile([128, nch, H], F32, tag="z")
                    nc.vector.tensor_tensor(
                        out=z[:], in0=gt[:, :, COL_ALS:COL_ALS + H],
                        in1=aldp[:], op=mybir.AluOpType.add)
                    e1 = zpool.tile([128, nch, H], F32, tag="e1")
                    nc.scalar.activation(out=e1[:], in_=z[:],
                                         func=mybir.ActivationFunctionType.Exp)
                    e2 = zpool.tile([128, nch, H], F32, tag="e2")
                    nc.scalar.activation(out=e2[:], in_=z[:],
                                         func=mybir.ActivationFunctionType.Exp,
                                         scale=float(NEG_SLOPE))
                    ah = zpool.tile([128, nch, H], F32, tag="ah")
                    nc.vector.tensor_max(out=ah[:], in0=e1[:], in1=e2[:])
                    xs = xpool.tile([128, nch, H, AGG_W], BF16, tag="xs")
                    for h in range(H):
                        nc.vector.tensor_tensor(
                            out=xs[:, :, h, :], in0=gt[:, :, 0:AGG_W],
                            in1=ah[:, :, h].unsqueeze(2)
                            .broadcast_to([128, nch, AGG_W]),
                            op=mybir.AluOpType.mult)

                for b in g["blocks"]:
                    idxs = by_block.get(b, [])
                    aggp = None
                    if idxs:
                        aggp = agps.tile([128, H, AGG_W], F32, tag="aggp")
                        for j, i in enumerate(idxs):
                            nc.tensor.matmul(
                                out=aggp[:], lhsT=mts[:, i, :],
                                rhs=xs[:, i, :, :],
                                start=(j == 0), stop=(j == len(idxs) - 1))
                    # ---- per-block post: self-loop fold + h1T + h2 ----
                    zs = qpool.tile([128, H], F32, tag="zs")
                    nc.vector.tensor_tensor(out=zs[:], in0=aldloc[:, b, 0:H],
                                            in1=aldloc[:, b, H:2 * H],
                                            op=mybir.AluOpType.add)
                    es1 = qpool.tile([128, H], F32, tag="es1")
                    nc.scalar.activation(out=es1[:], in_=zs[:],
                                         func=mybir.ActivationFunctionType.Exp)
                    es2 = qpool.tile([128, H], F32, tag="es2")
                    nc.scalar.activation(out=es2[:], in_=zs[:],
                                         func=mybir.ActivationFunctionType.Exp,
                                         scale=float(NEG_SLOPE))
                    aself = qpool.tile([128, H], F32, tag="aself")
                    nc.vector.tensor_max(out=aself[:], in0=es1[:], in1=es2[:])
                    den = qpool.tile([128, H], F32, tag="den")
                    if aggp is not None:
                        nc.vector.tensor_tensor(out=den[:],
                                                in0=aggp[:, :, COL_ONE],
                                                in1=aself[:],
                                                op=mybir.AluOpType.add)
                    else:
                        den = aself
                    rec = qpool.tile([128, H], F32, tag="rec")
                    nc.vector.reciprocal(out=rec[:], in_=den[:])
                    axn = qpool.tile([128, H, NODE_DIM], F32, tag="axn")
                    if aggp is not None:
                        for h in range(H):
                            nc.vector.scalar_tensor_tensor(
                                out=axn[:, h, :], in0=xloc_sb[:, b, :],
                                scalar=aself[:, h:h + 1],
                                in1=aggp[:, h, 0:NODE_DIM],
                                op0=mybir.AluOpType.mult,
                                op1=mybir.AluOpType.add)
                    else:
                        for h in range(H):
                            nc.vector.tensor_scalar(
                                out=axn[:, h, :], in0=xloc_sb[:, b, :],
                                scalar1=aself[:, h:h + 1], scalar2=None,
                                op0=mybir.AluOpType.mult)
                    axnb = qpool.tile([128, H, NODE_DIM], BF16, tag="axnb")
                    nc.vector.tensor_tensor(
                        out=axnb[:], in0=axn[:],
                        in1=rec[:].unsqueeze(2)
                        .broadcast_to([128, H, NODE_DIM]),
                        op=mybir.AluOpType.mult)
                    h1Tp = h1ps.tile([128, 8, 128], F32, tag="h1Tp")
                    for h in range(H):
                        axTp = trps.tile([NODE_DIM, 128], BF16, tag="axTp")
                        nc.tensor.transpose(out=axTp[:], in_=axnb[:, h, :],
                                            identity=identb[:])
                        axT = qpool.tile([NODE_DIM, 128], BF16, tag="axT")
                        nc.scalar.activation(
                            out=axT[:], in_=axTp[:],
                            func=mybir.ActivationFunctionType.Copy)
                        for sh in range(2):
                            s = 2 * h + sh
                            nc.tensor.matmul(
                                out=h1Tp[:, s, :],
                                lhsT=w1h[:, 128 * s:128 * (s + 1)],
                                rhs=axT[:], start=True, stop=True)
                    h1T = qpool.tile([128, 8, 128], BF16, tag="h1T")
                    for s in range(8):
                        nc.scalar.activation(
                            out=h1T[:, s, :], in_=h1Tp[:, s, :],
                            func=mybir.ActivationFunctionType.Relu)
                    h2p = h2ps.tile([128, HIDDEN + 2], F32, tag="h2p")
                    for s in range(8):
                        nc.tensor.matmul(out=h2p[:], lhsT=h1T[:, s, :],
                                         rhs=w2e[:, s, :],
                                         start=(s == 0), stop=(s == 7))
                    nc.scalar.activation(out=h2loc_sb[:, b, :],
                                         in_=h2p[:, 0:HIDDEN],
                                         func=mybir.ActivationFunctionType.Copy)
                    nc.vector.tensor_copy(out=al2loc[:, b, :],
                                          in_=h2p[:, HIDDEN:HIDDEN + 2])
                    nc.sync.dma_start(out=h2e_loc[128 * b:128 * (b + 1), :],
                                      in_=h2loc_sb[:, b, :])

        nc.gpsimd.collective_compute(
            "AllGather", mybir.AluOpType.bypass, replica_groups=replica,
            ins=[h2e_loc[:]], outs=[h2e_full[:]])

        # ------------------------------------------------------------------
        # Layer 2 + pooling accumulation
        resp_ctx = tc.tile_pool(name="resident_ps", bufs=1, space="PSUM")
        resp = resp_ctx.__enter__()
        poolp = resp.tile([NGL, HIDDEN + 1], F32, name="poolp", tag="poolp")
        with tc.tile_pool(name="g2_sb", bufs=2) as gpool, \
             tc.tile_pool(name="d2_sb", bufs=2) as dpool, \
             tc.tile_pool(name="m2_sb", bufs=2) as mpool, \
             tc.tile_pool(name="n2_sb", bufs=2) as npool, \
             tc.tile_pool(name="x2_sb", bufs=1) as xpool, \
             tc.tile_pool(name="z2_sb", bufs=2) as zpool, \
             tc.tile_pool(name="s2_sb", bufs=1) as spool, \
             tc.tile_pool(name="q2_sb", bufs=3) as qpool, \
             tc.tile_pool(name="ad2_ps", bufs=2, space="PSUM") as adps, \
             tc.tile_pool(name="ag2_ps", bufs=2, space="PSUM") as agps:

            nblk_done = [0]
            for g in lay.groups:
                nch = len(g["chunks"])
                t0 = g["t0"]
                by_block = {}
                for i, (b, kind) in enumerate(g["chunks"]):
                    by_block.setdefault(b, []).append(i)

                if nch:
                    gt = gpool.tile([128, nch, HW2], BF16, tag="gt2")
                    for r0, r1, base in _gather_runs(g, lay):
                        n = (r1 - r0) * 128
                        nc.gpsimd.dma_gather(
                            out_ap=gt[:, r0:r1, :],
                            in_ap=h2e_full[base:, :],
                            idxs_ap=gidx[:, (t0 + r0) * 8:(t0 + r1) * 8],
                            num_idxs=n, num_idxs_reg=n, elem_size=HW2)
                    dsr = dpool.tile([128, nch, 128], BF16, tag="dsr2")
                    nc.sync.dma_start(
                        out=dsr[:],
                        in_=dstrep_d[:, t0 * 128:(t0 + nch) * 128]
                        .rearrange("a (b c) -> a b c", c=128)
                        .broadcast_to([128, nch, 128]))
                    mts = mpool.tile([128, nch, 128], BF16, tag="mts2")
                    nc.vector.tensor_tensor(
                        out=mts[:],
                        in0=iota[:].unsqueeze(1).broadcast_to([128, nch, 128]),
                        in1=dstloc[:, t0:t0 + nch].unsqueeze(2)
                        .broadcast_to([128, nch, 128]),
                        op=mybir.AluOpType.is_equal)
                    mns = npool.tile([128, nch, 128], BF16, tag="mns2")
                    nc.vector.tensor_scalar(
                        out=mns[:], in0=dsr[:], scalar1=iotac[:], scalar2=None,
                        op0=mybir.AluOpType.is_equal)
                    scr = spool.tile([128, nch, HW2], BF16, tag="scr")
                    nc.vector.tensor_tensor(
                        out=scr[:], in0=gt[:],
                        in1=a2srep[:].unsqueeze(1)
                        .broadcast_to([128, nch, HW2]),
                        op=mybir.AluOpType.mult)
                    als2e = zpool.tile([128, nch], F32, tag="als2e")
                    nc.vector.tensor_reduce(
                        out=als2e[:], in_=scr[:],
                        axis=mybir.AxisListType.X, op=mybir.AluOpType.add)
                    aldp = adps.tile([128, nch, 1], F32, tag="aldp2")
                    for i, (b, kind) in enumerate(g["chunks"]):
                        nc.tensor.matmul(out=aldp[:, i, :], lhsT=mns[:, i, :],
                                         rhs=al2loc[:, b, 1:2],
                                         start=True, stop=True)
                    z = zpool.tile([128, nch], F32, tag="z2")
                    nc.vector.tensor_tensor(out=z[:], in0=als2e[:],
                                            in1=aldp[:, :, 0],
                                            op=mybir.AluOpType.add)
                    e1 = zpool.tile([128, nch], F32, tag="e12")
                    nc.scalar.activation(out=e1[:], in_=z[:],
                                         func=mybir.ActivationFunctionType.Exp)
                    e2 = zpool.tile([128, nch], F32, tag="e22")
                    nc.scalar.activation(out=e2[:], in_=z[:],
                                         func=mybir.ActivationFunctionType.Exp,
                                         scale=float(NEG_SLOPE))
                    ah = zpool.tile([128, nch], F32, tag="ah2")
                    nc.vector.tensor_max(out=ah[:], in0=e1[:], in1=e2[:])
                    xs = xpool.tile([128, nch, HW2 + 1], BF16, tag="xs2")
                    nc.vector.tensor_tensor(
                        out=xs[:, :, 0:HW2], in0=gt[:],
                        in1=ah[:].unsqueeze(2).broadcast_to([128, nch, HW2]),
                        op=mybir.AluOpType.mult)
                    nc.vector.tensor_copy(out=xs[:, :, HW2], in_=ah[:])

                for b in g["blocks"]:
                    idxs = by_block.get(b, [])
                    aggp = None
                    if idxs:
                        aggp = agps.tile([128, HW2 + 1], F32, tag="aggp2")
                        for j, i in enumerate(idxs):
                            nc.tensor.matmul(
                                out=aggp[:], lhsT=mts[:, i, :], rhs=xs[:, i, :],
                                start=(j == 0), stop=(j == len(idxs) - 1))
                    # ---- per-block post: self fold + relu + pool matmul ----
                    zs = qpool.tile([128, 1], F32, tag="zs2")
                    nc.vector.tensor_tensor(out=zs[:], in0=al2loc[:, b, 0:1],
                                            in1=al2loc[:, b, 1:2],
                                            op=mybir.AluOpType.add)
                    es1 = qpool.tile([128, 1], F32, tag="es12")
                    nc.scalar.activation(out=es1[:], in_=zs[:],
                                         func=mybir.ActivationFunctionType.Exp)
                    es2 = qpool.tile([128, 1], F32, tag="es22")
                    nc.scalar.activation(out=es2[:], in_=zs[:],
                                         func=mybir.ActivationFunctionType.Exp,
                                         scale=float(NEG_SLOPE))
                    aself = qpool.tile([128, 1], F32, tag="aself2")
                    nc.vector.tensor_max(out=aself[:], in0=es1[:], in1=es2[:])
                    hf = qpool.tile([128, HIDDEN + 1], BF16, tag="hf")
                    if aggp is not None:
                        den = qpool.tile([128, 1], F32, tag="den2")
                        nc.vector.tensor_tensor(out=den[:],
                                                in0=aggp[:, HW2:HW2 + 1],
                                                in1=aself[:],
                                                op=mybir.AluOpType.add)
                        rec = qpool.tile([128, 1], F32, tag="rec2")
                        nc.vector.reciprocal(out=rec[:], in_=den[:])
                        numer = qpool.tile([128, HIDDEN], F32, tag="numer")
                        nc.vector.scalar_tensor_tensor(
                            out=numer[:], in0=h2loc_sb[:, b, :],
                            scalar=aself[:], in1=aggp[:, 0:HW2],
                            op0=mybir.AluOpType.mult, op1=mybir.AluOpType.add)
                        nc.vector.tensor_scalar(
                            out=hf[:, 0:HIDDEN], in0=numer[:],
                            scalar1=rec[:], scalar2=0.0,
                            op0=mybir.AluOpType.mult, op1=mybir.AluOpType.max)
                    else:
                        nc.vector.tensor_scalar(
                            out=hf[:, 0:HIDDEN], in0=h2loc_sb[:, b, :],
                            scalar1=0.0, scalar2=None,
                            op0=mybir.AluOpType.max)
                    nc.vector.memset(hf[:, HIDDEN:HIDDEN + 1], 1.0)
                    nc.tensor.matmul(out=poolp[:], lhsT=mg_all[:, b, 0:NGL],
                                     rhs=hf[:],
                                     start=(nblk_done[0] == 0),
                                     stop=(nblk_done[0] == NB - 1))
                    nblk_done[0] += 1

        # ------------------------------------------------------------------
        # Mean + FC
        with tc.tile_pool(name="p5_sb", bufs=2) as p5s, \
             tc.tile_pool(name="p5_ps", bufs=2, space="PSUM") as p5p:
            cnt = p5s.tile([NGL, 1], F32, name="cnt")
            nc.vector.tensor_scalar(out=cnt[:], in0=poolp[:, HIDDEN:HIDDEN + 1],
                                    scalar1=1.0, scalar2=None,
                                    op0=mybir.AluOpType.max)
            crec = p5s.tile([NGL, 1], F32, name="crec")
            nc.vector.reciprocal(out=crec[:], in_=cnt[:])
            pm = p5s.tile([NGL, HIDDEN], BF16, name="pm")
            nc.vector.tensor_scalar(out=pm[:], in0=poolp[:, 0:HIDDEN],
                                    scalar1=crec[:], scalar2=None,
                                    op0=mybir.AluOpType.mult)
            fcp = p5p.tile([NGL, OUT_DIM], F32, name="fcp")
            for k in range(2):
                pmTp = p5p.tile([128, NGL], BF16, tag="pmTp")
                nc.tensor.transpose(out=pmTp[:], in_=pm[:, 128 * k:128 * (k + 1)],
                                    identity=identb[0:NGL, 0:NGL])
                pmT = p5s.tile([128, NGL], BF16, tag="pmT")
                nc.scalar.activation(out=pmT[:], in_=pmTp[:],
                                     func=mybir.ActivationFunctionType.Copy)
                nc.tensor.matmul(out=fcp[:], lhsT=pmT[:], rhs=fcw[:, k, :],
                                 start=(k == 0), stop=(k == 1))
            outs = p5s.tile([NGL, OUT_DIM], F32, name="outs")
            nc.vector.tensor_copy(out=outs[:], in_=fcp[:])
            nc.sync.dma_start(out=out_d[:], in_=outs[:])

        resp_ctx.__exit__(None, None, None)
        res_ctx.__exit__(None, None, None)

    nc.compile()
    return nc


def _gather_runs(g, lay):
    """Split a group's chunk list into dma_gather calls: same lo/hi kind,
    contiguous chunk positions, <= MAX_GATHER_CHUNKS per call."""
    nch = len(g["chunks"])
    runs = []
    r0 = 0
    while r0 < nch:
        kind = g["chunks"][r0][1]
        r1 = r0 + 1
        while (r1 < nch and r1 - r0 < MAX_GATHER_CHUNKS
               and g["chunks"][r1][1] == kind):
            r1 += 1
        runs.append((r0, r1, 0 if kind == "lo" else lay.HI_BASE))
        r0 = r1
    return runs


def prep_inputs(x, edge_index, batch, W1, a_src1, a_dst1, b1, W2, a_src2,
                a_dst2, b2, fc_W, fc_b, n_cores=N_CORES, group_blocks=8):
    """Host-side: shard + pack all per-core input tensors."""
    n = x.shape[0]
    src = np.asarray(edge_index[0]).astype(np.int64)
    dst = np.asarray(edge_index[1]).astype(np.int64)
    batch = np.asarray(batch).astype(np.int64)
    x = np.asarray(x, dtype=np.float32)

    lay = Layout(n, int(batch.max()) + 1, n_cores, src, dst, batch,
                 group_blocks=group_blocks)

    bf = ml_dtypes.bfloat16
    W1 = np.asarray(W1, np.float32)
    was1 = np.einsum("dhk,hk->dh", W1.reshape(NODE_DIM, HEADS, HIDDEN),
                     np.asarray(a_src1, np.float32))
    wad1 = np.einsum("dhk,hk->dh", W1.reshape(NODE_DIM, HEADS, HIDDEN),
                     np.asarray(a_dst1, np.float32))
    wasd1 = np.concatenate([was1, wad1], axis=1).astype(bf)
    W2 = np.asarray(W2, np.float32)
    w2e = np.concatenate([
        W2,
        (W2 @ np.asarray(a_src2, np.float32)[0])[:, None],
        (W2 @ np.asarray(a_dst2, np.float32)[0])[:, None],
    ], axis=1).astype(bf)
    NGP = int(math.ceil(lay.g_per_core / 32) * 32)
    iota128 = np.tile(np.arange(128, dtype=np.float32), (128, 1)).astype(bf)
    iotacol = np.arange(128, dtype=np.float32).reshape(128, 1)
    iotag = np.tile(np.arange(NGP, dtype=np.float32), (128, 1)).astype(bf)
    identb = np.eye(128, dtype=np.float32).astype(bf)

    common = {
        "wasd1": wasd1,
        "w1h": W1.astype(bf),
        "w2e": w2e,
        "a2srow": np.asarray(a_src2, np.float32).reshape(1, HIDDEN).astype(bf),
        "fcw": np.asarray(fc_W, np.float32).astype(bf),
        "iota128": iota128,
        "iotacol": iotacol,
        "iotag": iotag,
        "identb": identb,
    }
    in_maps = []
    for c in range(n_cores):
        gidx, dstloc, dstrep = lay.pack_core(c, src, dst)
        xl = np.zeros((lay.NLOC, NODE_DIM), dtype=np.float32)
        ns, ne = lay.core_start[c], lay.core_end[c]
        xl[: ne - ns] = x[ns:ne]
        m = dict(common)
        m["xloc"] = xl.astype(bf)
        m["gidx"] = gidx
        m["dstloc"] = dstloc
        m["dstrep"] = dstrep
        m["batchloc"] = lay.pack_batchloc(c, batch)
        in_maps.append(m)
    return lay, in_maps


def kernel(**inputs) -> np.ndarray:
    lay, in_maps = prep_inputs(**inputs)
    nc = build_program(lay, N_CORES)
    res = run_bass_kernel_spmd(nc, in_maps, list(range(N_CORES)))
    outs = [np.asarray(res.results[c]["out"], dtype=np.float32)
            for c in range(N_CORES)]
    return np.concatenate(outs, axis=0)
